# revision 36
# baseline (speedup 1.0000x reference)
"""FastVAR cross-attention block kernel for 8 Trainium2 NeuronCores.

Sharding: 2 batches x 4 head-groups (4 heads each) = 8 cores (SPMD).

Per-core device program (v3 — engine-balanced, low DMA-dispatch count):
  phase 1 (per 128-token chunk; one 3D-AP DMA per chunk):
    qkv matmul (bf16, fp32 psum); norms/normalize read PSUM directly on DVE
    RoPE in split-halves layout (even|odd): 4 muls on GpSimd, 2 combines on DVE
    v(+ones) packed by ACT; q,k transposed on PE (+ACT copy) to feature-major
  phase 2 (attention, 8 units = 2 head-pairs x 4 query-quarters of 416):
    flat software pipeline across (unit, kb) steps: scores for step i+1 are
    emitted before AV of step i so the in-order PE queue never blocks ACT
    scores: 2 row-tiled concurrent K=64 matmuls -> one 2-bank PSUM tile
    exp: single ACT instruction over both heads (N=832) -> bf16 SBUF
    AV with ones-augmented V (M=65) accumulating over k-chunks in PSUM
    normalize: early PSUM->SBUF copy, fast reciprocal, one DRAM broadcast
    projection interleaved one half-chunk per step (PSUM-pool reuse)
Host: top-k selection (bitwise-matches reference on CPU jax), gather, weight
slicing (rope even|odd permutation folded into W_qkv), partial-sum reduce,
scatter + residual.
"""

import math
import os
import sys
from contextlib import ExitStack

import numpy as np

import concourse.bass as bass
import concourse.bacc as bacc
import concourse.tile as tile
from concourse import mybir
from concourse import bass_utils

# ---------------------------------------------------------------- constants
B = 2
L = 4096
C = 1024
NH = 16
DH = 64
NREM = 1638          # num_remain for this problem
NT = 1664            # padded token count (13 * 128)
HPG = 4              # heads per core (16 heads / 4 groups)
N_CORES = 8
QQ = 416             # query block (4 blocks of 416 = 1664)

F32 = mybir.dt.float32
BF16 = mybir.dt.bfloat16


class Cfg:
    def __init__(self, NT, NTR, C, HPG, DH, has_bias=True):
        self.NT, self.NTR, self.C, self.HPG, self.DH = NT, NTR, C, HPG, DH
        self.has_bias = has_bias
        self.NC = NT // 128          # token chunks
        self.CH = C // 128           # contraction chunks
        self.F = 3 * HPG * DH        # qkv feature width (768)
        self.HC = HPG * DH           # head channels per core (256)
        self.HCC = self.HC // 128    # proj contraction chunks (2)


FULL_CFG = Cfg(NT=NT, NTR=NREM, C=C, HPG=HPG, DH=DH)


# ---------------------------------------------------------------- device IR
def emit_core_program(tc, outs, ins, cfg):
    nc = tc.nc
    NTc, NC, CH = cfg.NT, cfg.NC, cfg.CH
    F = cfg.F
    X = mybir.AxisListType.X
    AF = mybir.ActivationFunctionType

    xmT, wqkvT = ins["xmT"], ins["wqkvT"]
    ropeC, ropeS = ins["ropeC"], ins["ropeS"]
    wpT, scales = ins["wpT"], ins["scales"]
    outp = outs["outp"]

    with ExitStack() as ctx:
        const = ctx.enter_context(tc.tile_pool(name="const", bufs=1))

        # xm per-chunk tiles; one 3D DMA per token chunk (chunk-level deps)
        xm_c = [const.tile([128, CH, 128], BF16, name=f"xmc{t}",
                           tag=f"xmc{t}") for t in range(NC)]
        xmT_r = xmT[0:cfg.C, :].rearrange("(c p) t -> p c t", p=128)
        # first chunks before the weights so chunk 0 can start early
        for t in range(4):
            nc.sync.dma_start(xm_c[t][:], xmT_r[:, :, t * 128:(t + 1) * 128])

        # ---- resident input tiles ----------------------------------------
        w_t = []
        for ci in range(CH):
            t = const.tile([128, F], BF16, tag=f"w{ci}")
            nc.sync.dma_start(t[:], wqkvT[ci * 128:(ci + 1) * 128, :])
            w_t.append(t)
        w_bias = const.tile([1, F], BF16, tag="wb")
        nc.sync.dma_start(w_bias[:], wqkvT[cfg.C:cfg.C + 1, :])
        wp_t = []
        for hc in range(cfg.HCC):
            t = const.tile([128, cfg.C], BF16, tag=f"wp{hc}")
            nc.sync.dma_start(t[:], wpT[hc * 128:(hc + 1) * 128, :])
            wp_t.append(t)
        s_t = const.tile([128, cfg.HPG], F32, tag="scales")
        nc.sync.dma_start(s_t[:], scales[0:1, :].to_broadcast((128, cfg.HPG)))
        # rope tables, chunk-major: [128, NC, DH//2] (one DMA each)
        rct = const.tile([128, NC, DH // 2], BF16, tag="rct")
        rst = const.tile([128, NC, DH // 2], BF16, tag="rst")
        nc.sync.dma_start(rct[:], ropeC.rearrange("(c p) d -> p c d", p=128))
        nc.sync.dma_start(rst[:], ropeS.rearrange("(c p) d -> p c d", p=128))
        ones_row = const.tile([1, NTc], BF16, tag="ones_row")
        nc.sync.dma_start(ones_row[:], xmT[cfg.C:cfg.C + 1, :])
        ident = const.tile([128, 128], BF16, tag="ident")
        nc.sync.dma_start(ident[:], ins["ident"][:])
        eps_t = const.tile([128, 1], F32, tag="eps")
        nc.vector.memset(eps_t[:], 1e-12)

        # feature-major q,k (pairs of heads stacked 64+64)
        qkT = const.tile([128, 4, NTc], BF16, tag="qkT")
        vav = const.tile([128, NC, cfg.HPG, DH + 1], BF16, tag="vav")
        oPair = [const.tile([128, NTc], BF16, name=f"oP{i}", tag=f"oP{i}")
                 for i in range(2)]

        wk = ctx.enter_context(tc.tile_pool(name="wk", bufs=2))
        gp = ctx.enter_context(tc.tile_pool(name="gp", bufs=2))
        pe = ctx.enter_context(tc.tile_pool(name="exp", bufs=3))
        ms = ctx.enter_context(tc.tile_pool(name="ms", bufs=2))

        # ones column of vav is constant: set once (zero pad rows of last chunk)
        pad0 = cfg.NTR - (NC - 1) * 128
        nc.vector.memset(vav[:, 0:NC - 1, :, DH:DH + 1], 1.0)
        nc.vector.memset(vav[:, NC - 1, :, DH:DH + 1], 0.0)
        nc.vector.memset(vav[0:pad0, NC - 1, :, DH:DH + 1], 1.0)

        # ---------------- phase 1: qkv + norm + rope + transposes ----------
        with tc.tile_pool(name="p1ps", bufs=3, space="PSUM") as p1, \
             tc.tile_pool(name="tpps", bufs=2, space="PSUM") as tp:
            tp_pending = []

            def drain_tp(final=False):
                # keep the newest entry queued: its qkr is still ~1.4us from
                # ready when this chunk's matmuls end (2-chunk deferral)
                todo = tp_pending if final else tp_pending[:-1]
                for dsl, src in todo:
                    tps = tp.tile([128, 4, 128], BF16, name="tps", tag="tps")
                    for j in range(4):
                        nc.tensor.transpose(
                            tps[:, j, :], src[:, j * 128:(j + 1) * 128],
                            ident[:])
                    nc.scalar.copy(qkT[:, :, dsl], tps[:])
                tp_pending[:] = [] if final else tp_pending[-1:]

            for t in range(NC):
                tsl = slice(t * 128, (t + 1) * 128)
                if t + 4 < NC:
                    t2 = t + 4
                    nc.sync.dma_start(xm_c[t2][:],
                                      xmT_r[:, :, t2 * 128:(t2 + 1) * 128])

                ps = p1.tile([128, F], F32)
                n_ci = CH + 1 if cfg.has_bias else CH
                for ci in range(n_ci):
                    lhs = xm_c[t][:, ci, :] if ci < CH else ones_row[:, tsl]
                    rhsw = w_t[ci] if ci < CH else w_bias
                    for n0 in range(0, F, 512):
                        nn = min(512, F - n0)
                        nc.tensor.matmul(
                            ps[:, n0:n0 + nn], lhs, rhsw[:, n0:n0 + nn],
                            start=(ci == 0), stop=(ci == n_ci - 1))
                # previous chunk's transposes go behind this chunk's matmuls
                # so the PE never waits on the norm/rope chain
                drain_tp()

                # l2 norms over dh for the 8 q+k head-groups (PSUM-direct)
                sq = wk.tile([128, 512], BF16, tag="sq")
                nc.scalar.square(sq[:], ps[:, 0:512])
                ss = wk.tile([128, 8], F32, tag="ss")
                nc.vector.reduce_sum(
                    ss[:], sq.rearrange("p (h d) -> p h d", d=DH), axis=X)
                sroot = wk.tile([128, 8], F32, tag="sroot")
                nc.scalar.activation(sroot[:], ss[:], AF.Sqrt,
                                     bias=eps_t[:])
                rr8 = wk.tile([128, 8, 1], F32, tag="rr8")
                nc.vector.reciprocal(
                    rr8.rearrange("p h one -> p (h one)"), sroot[:])
                rrq = wk.tile([128, cfg.HPG, 1], F32, tag="rrq")
                nc.vector.tensor_mul(
                    rrq.rearrange("p h one -> p (h one)"),
                    rr8[:, 0:cfg.HPG, 0], s_t[:])

                # normalize q (with head scale) and k, direct from PSUM
                qkn = wk.tile([128, 8, DH], BF16, tag="qkn")
                nc.vector.tensor_mul(
                    qkn[:, 0:4, :],
                    ps[:, 0:256].rearrange("p (h d) -> p h d", d=DH),
                    rrq.to_broadcast((128, 4, DH)))
                nc.vector.tensor_mul(
                    qkn[:, 4:8, :],
                    ps[:, 256:512].rearrange("p (h d) -> p h d", d=DH),
                    rr8[:, 4:8, :].to_broadcast((128, 4, DH)))

                # v -> vav (ACT copy, PSUM source)
                nc.scalar.copy(
                    vav[:, t, :, 0:DH],
                    ps[:, 512:768].rearrange("p (h d) -> p h d", d=DH))

                # rope (dh layout = [even32 | odd32] per head): muls on gpsimd
                DH2 = DH // 2
                rc_b = rct[:, t:t + 1, :].to_broadcast((128, 8, DH2))
                rs_b = rst[:, t:t + 1, :].to_broadcast((128, 8, DH2))
                m_ce = gp.tile([128, 8, DH2], BF16, tag="m_ce")
                m_so = gp.tile([128, 8, DH2], BF16, tag="m_so")
                m_se = gp.tile([128, 8, DH2], BF16, tag="m_se")
                m_co = gp.tile([128, 8, DH2], BF16, tag="m_co")
                nc.gpsimd.tensor_mul(m_ce[:], qkn[:, :, 0:DH2], rc_b)
                nc.gpsimd.tensor_mul(m_so[:], qkn[:, :, DH2:DH], rs_b)
                nc.gpsimd.tensor_mul(m_se[:], qkn[:, :, 0:DH2], rs_b)
                nc.gpsimd.tensor_mul(m_co[:], qkn[:, :, DH2:DH], rc_b)
                qkr = wk.tile([128, 8, DH], BF16, tag="qkr", bufs=3)
                nc.vector.tensor_sub(qkr[:, :, 0:DH2], m_ce[:], m_so[:])
                nc.vector.tensor_add(qkr[:, :, DH2:DH], m_se[:], m_co[:])

                # feature-major q,k via PE transpose + ACT copy (deferred)
                qkr2 = qkr.rearrange("p h d -> p (h d)")
                tp_pending.append((tsl, qkr2))
            drain_tp(final=True)

        # ---------------- phase 2: attention + interleaved projection ------
        # 3 main query blocks of 512 (bank-exact) + one shared 128-query tail
        # unit covering all 4 heads. Scores for step i+1 are emitted before AV
        # of step i; projection drains one matmul per step.
        QB = 512
        QT0 = 3 * QB                       # tail start (1536)
        steps = [("m", qq, pair, kb)
                 for qq in range(3) for pair in range(2) for kb in range(NC)]
        steps += [("t", 0, 0, kb) for kb in range(NC)]

        with tc.tile_pool(name="scps", bufs=2, space="PSUM") as scp, \
             tc.tile_pool(name="otps", bufs=1, space="PSUM") as otp, \
             tc.tile_pool(name="pjps", bufs=2, space="PSUM") as pjp, \
             tc.tile_pool(name="dscr", bufs=4, space="DRAM") as pd:

            sc_tiles = {}
            oT_cur = [None, None]
            proj_pending = []   # single-matmul granularity
            proj_state = {}

            def emit_scores(step):
                kind, qq, pair, kb = step
                ksl = slice(kb * 128, (kb + 1) * 128)
                scP = scp.tile([128, 2, QB], F32, tag="sc")
                if kind == "m":
                    qsl = slice(qq * QB, (qq + 1) * QB)
                    for i in range(2):
                        nc.tensor.matmul(
                            scP[:, i, :],
                            qkT[i * DH:(i + 1) * DH, 2 + pair, ksl],
                            qkT[i * DH:(i + 1) * DH, pair, qsl],
                            start=True, stop=True)
                else:
                    # row-tile index i selects the bank so the two concurrent
                    # matmuls never write the same PSUM bank
                    for h in range(4):
                        p2_, i = h // 2, h % 2
                        nc.tensor.matmul(
                            scP[:, i, p2_ * 256:p2_ * 256 + 128],
                            qkT[i * DH:(i + 1) * DH, 2 + p2_, ksl],
                            qkT[i * DH:(i + 1) * DH, p2_, QT0:NTc],
                            start=True, stop=True)
                sc_tiles[step] = scP

            def emit_proj_one(end_drain=False):
                """Emit a single projection matmul (plus copy+store when a
                half-chunk completes)."""
                if not proj_pending:
                    return
                t, half = proj_pending[0]
                hc = proj_state.get((t, half), 0)
                tsl = slice(t * 128, (t + 1) * 128)
                nsl = slice(half * 512, (half + 1) * 512)
                if hc == 0:
                    psp = pjp.tile([128, 512], F32, tag="pj")
                    proj_state[(t, half, "ps")] = psp
                else:
                    psp = proj_state.pop((t, half, "ps"))
                nc.tensor.matmul(
                    psp[:], oPair[hc][:, tsl], wp_t[hc][:, nsl],
                    start=(hc == 0), stop=(hc == cfg.HCC - 1))
                if hc == cfg.HCC - 1:
                    ob = ms.tile([128, 512], BF16, tag="ob")
                    if end_drain:
                        # ACT is idle after the last exp; the DVE FIFO is
                        # blocked behind the tail-unit reciprocal's DMA wait
                        nc.scalar.copy(ob[:], psp[:])
                    else:
                        nc.vector.tensor_copy(ob[:], psp[:])
                    nc.gpsimd.dma_start(outp[tsl, nsl], ob[:])
                    proj_pending.pop(0)
                    proj_state.pop((t, half), None)
                else:
                    proj_state[(t, half)] = hc + 1

            def normalize_main(pair, qq):
                qsl = slice(qq * QB, (qq + 1) * QB)
                oTs = []
                for i in range(2):
                    t_sb = ms.tile([DH + 1, QB], F32, name=f"oTs{i}",
                                   tag=f"oTs{i}")
                    nc.vector.tensor_copy(t_sb[:], oT_cur[i][:])
                    oTs.append(t_sb)
                den_d = pd.tile([1, 2 * QB], F32, name="den_d", tag="den_d")
                for i in range(2):
                    nc.sync.dma_start(den_d[0:1, i * QB:(i + 1) * QB],
                                      oTs[i][DH:DH + 1, :])
                NP8 = 2 * QB // 8
                dshuf = bass.AP(tensor=den_d.tensor, offset=den_d.offset,
                                ap=[[8, NP8], [1, 8]])
                dt = ms.tile([NP8, 8], F32, tag="dt")
                nc.sync.dma_start(dt[:], dshuf)
                rt = ms.tile([NP8, 8], F32, tag="rt")
                nc.vector.reciprocal(rt[:], dt[:])
                rec_d = pd.tile([1, 2 * QB], F32, name="rec_d", tag="rec_d")
                rshuf = bass.AP(tensor=rec_d.tensor, offset=rec_d.offset,
                                ap=[[8, NP8], [1, 8]])
                nc.sync.dma_start(rshuf, rt[:])
                reb = ms.tile([DH, 2, QB], F32, tag="reb")
                bc_src = bass.AP(tensor=rec_d.tensor, offset=rec_d.offset,
                                 ap=[[0, DH], [1, 2 * QB]])
                nc.sync.dma_start(
                    reb.rearrange("p two q -> p (two q)"), bc_src)
                for i in range(2):
                    nc.vector.tensor_mul(
                        oPair[pair][i * DH:(i + 1) * DH, qsl],
                        oTs[i][0:DH, :], reb[:, i, :])

            def normalize_tail():
                oTs = []
                for i in range(2):
                    t_sb = ms.tile([DH + 1, QB], F32, name=f"oTs{i}",
                                   tag=f"oTs{i}")
                    nc.vector.tensor_copy(t_sb[:], oT_cur[i][:])
                    oTs.append(t_sb)
                den_d = pd.tile([1, 512], F32, name="den_dt", tag="den_dt")
                for p2_ in range(2):
                    src3 = oTs[p2_][DH:DH + 1, :].rearrange(
                        "one (j x) -> one j x", j=2)[:, :, 0:128]
                    dst3 = den_d[0:1, p2_ * 256:(p2_ + 1) * 256].rearrange(
                        "one (j x) -> one j x", x=128)
                    nc.sync.dma_start(dst3, src3)
                NP8 = 64
                dshuf = bass.AP(tensor=den_d.tensor, offset=den_d.offset,
                                ap=[[8, NP8], [1, 8]])
                dt = ms.tile([NP8, 8], F32, tag="dtt")
                nc.sync.dma_start(dt[:], dshuf)
                rt = ms.tile([NP8, 8], F32, tag="rtt")
                nc.vector.reciprocal(rt[:], dt[:])
                rec_d = pd.tile([1, 512], F32, name="rec_dt", tag="rec_dt")
                rshuf = bass.AP(tensor=rec_d.tensor, offset=rec_d.offset,
                                ap=[[8, NP8], [1, 8]])
                nc.sync.dma_start(rshuf, rt[:])
                reb = ms.tile([DH, 512], F32, tag="rebt")
                bc_src = bass.AP(tensor=rec_d.tensor, offset=rec_d.offset,
                                 ap=[[0, DH], [1, 512]])
                nc.sync.dma_start(reb[:], bc_src)
                for h in range(4):
                    p2_, i = h // 2, h % 2
                    nc.vector.tensor_mul(
                        oPair[p2_][i * DH:(i + 1) * DH, QT0:NTc],
                        oTs[p2_][0:DH, i * 256:i * 256 + 128],
                        reb[:, h * 128:(h + 1) * 128])

            emit_scores(steps[0])
            for si, step in enumerate(steps):
                kind, qq, pair, kb = step
                if kb == 0:
                    oT_cur = [otp.tile([DH + 1, QB], F32, name=f"oT{i}",
                                       tag=f"oT{i}") for i in range(2)]
                if si + 1 < len(steps):
                    emit_scores(steps[si + 1])
                scP = sc_tiles.pop(step)
                if kind == "m":
                    exb = pe.tile([128, 2, QB], BF16, tag="ex")
                    nc.scalar.activation(exb[:], scP[:], AF.Exp)
                    for i in range(2):
                        h = pair * 2 + i
                        nc.tensor.matmul(
                            oT_cur[i][:], vav[:, kb, h, :], exb[:, i, :],
                            start=(kb == 0), stop=(kb == NC - 1))
                else:
                    sc4 = scP.rearrange("p two (j x) -> p two j x",
                                        j=2)[:, :, :, 0:128]
                    exb = pe.tile([128, 2, 2, 128], BF16, tag="ext")
                    nc.scalar.activation(exb[:], sc4, AF.Exp)
                    # start=True clears the WHOLE bank's has_written bits,
                    # so only the first head writing each oT bank may set it
                    for h in range(4):
                        p2_, i = h // 2, h % 2
                        nc.tensor.matmul(
                            oT_cur[p2_][:, i * 256:i * 256 + 128],
                            vav[:, kb, h, :], exb[:, i, p2_, :],
                            start=(kb == 0 and i == 0),
                            stop=(kb == NC - 1))
                emit_proj_one()
                if kb == NC - 1:
                    if kind == "m":
                        normalize_main(pair, qq)
                        if pair == 1:
                            for t in range(4 * qq, 4 * qq + 4):
                                proj_pending.append((t, 0))
                                proj_pending.append((t, 1))
                    else:
                        # flush leftovers first: their DVE copies must not
                        # queue behind the tail reciprocal's DMA round-trip
                        while proj_pending:
                            emit_proj_one(end_drain=True)
                        normalize_tail()
                        proj_pending.append((12, 0))
                        proj_pending.append((12, 1))
            while proj_pending:
                emit_proj_one(end_drain=True)


# ---------------------------------------------------------------- build
def declare_io(nc, cfg):
    ins = {
        "xmT": nc.dram_tensor("xmT", [cfg.C + 1, cfg.NT], BF16,
                              kind="ExternalInput").ap(),
        "wqkvT": nc.dram_tensor("wqkvT", [cfg.C + 1, cfg.F], BF16,
                                kind="ExternalInput").ap(),
        "ropeC": nc.dram_tensor("ropeC", [cfg.NT, cfg.DH // 2], BF16,
                                kind="ExternalInput").ap(),
        "ropeS": nc.dram_tensor("ropeS", [cfg.NT, cfg.DH // 2], BF16,
                                kind="ExternalInput").ap(),
        "wpT": nc.dram_tensor("wpT", [cfg.HC, cfg.C], BF16,
                              kind="ExternalInput").ap(),
        "scales": nc.dram_tensor("scales", [1, cfg.HPG], F32,
                                 kind="ExternalInput").ap(),
        "ident": nc.dram_tensor("ident", [128, 128], BF16,
                                kind="ExternalInput").ap(),
    }
    outs = {
        "outp": nc.dram_tensor("outp", [cfg.NT, cfg.C], BF16,
                               kind="ExternalOutput").ap(),
    }
    return ins, outs


_BUILD_CACHE = {}

if os.environ.get("LDW_OPT", "0") == "1":
    _orig_run_command = bass_utils.run_command

    def _patched_run_command(argv, **kw):
        argv = ["--enable-ldw-opt=true" if a == "--enable-ldw-opt=false" else a
                for a in argv]
        return _orig_run_command(argv, **kw)

    bass_utils.run_command = _patched_run_command


def build_full_program(has_bias=True):
    key = ("full", has_bias)
    if key in _BUILD_CACHE:
        return _BUILD_CACHE[key]
    cfg = Cfg(NT=NT, NTR=NREM, C=C, HPG=HPG, DH=DH, has_bias=has_bias)
    nc = bacc.Bacc("TRN2", target_bir_lowering=False, debug=False,
                   num_devices=N_CORES)
    ins, outs = declare_io(nc, cfg)
    with tile.TileContext(nc) as tc:
        emit_core_program(tc, outs, ins, cfg)
    nc.compile()
    _BUILD_CACHE[key] = nc
    return nc


# ---------------------------------------------------------------- host side
def _topk_idx(x, n):
    """Replicate reference token selection exactly (CPU jax; numpy fallback)."""
    try:
        import jax
        import jax.numpy as jnp
        cpu = jax.devices("cpu")[0]
        with jax.default_device(cpu):
            xj = jax.device_put(np.asarray(x), cpu)
            mean = jnp.mean(xj, axis=1, keepdims=True)
            mse = jnp.sum((xj - mean) ** 2, axis=-1)
            idx = jnp.argsort(-mse, axis=1)[:, :n]
            return np.asarray(idx)
    except Exception:
        x = np.asarray(x, np.float32)
        mean = x.mean(1, keepdims=True, dtype=np.float32)
        mse = ((x - mean) ** 2).sum(-1, dtype=np.float32)
        return np.argsort(-mse, axis=1, kind="stable")[:, :n]


# rope even|odd permutation of a head's 64 features
_PERM = np.concatenate([np.arange(0, DH, 2), np.arange(1, DH, 2)])


def make_in_maps(x, cached_x, W_qkv, q_bias, v_bias, W_proj, b_proj,
                 scale_mul_log, rope_grid, idx, cfg):
    x = np.asarray(x, np.float32)
    W_qkv = np.asarray(W_qkv, np.float32)
    W_proj = np.asarray(W_proj, np.float32)
    q_bias = np.asarray(q_bias, np.float32)
    v_bias = np.asarray(v_bias, np.float32)
    rope_grid = np.asarray(rope_grid, np.float32)
    scale = np.exp(np.minimum(np.asarray(scale_mul_log, np.float32),
                              math.log(100.0))).reshape(NH)

    n_groups = NH // cfg.HPG
    in_maps = []
    per_batch = {}
    import ml_dtypes
    bf = ml_dtypes.bfloat16
    for b in range(B):
        xm = x[b, idx[b]]                                   # (NREM, C)
        xmT = np.zeros((cfg.C + 1, cfg.NT), np.float32)
        xmT[:cfg.C, :cfg.NTR] = xm.T
        xmT[cfg.C, :cfg.NTR] = 1.0
        ropeC = np.zeros((cfg.NT, DH // 2), np.float32)
        ropeS = np.zeros((cfg.NT, DH // 2), np.float32)
        ropeC[:cfg.NTR] = rope_grid[0][idx[b]]
        ropeS[:cfg.NTR] = rope_grid[1][idx[b]]
        per_batch[b] = (xmT.astype(bf), ropeC.astype(bf), ropeS.astype(bf))

    for core in range(N_CORES):
        b, hg = divmod(core, n_groups)
        hs = list(range(hg * cfg.HPG, (hg + 1) * cfg.HPG))
        xmT, ropeC, ropeS = per_batch[b]

        wq = np.zeros((cfg.C + 1, cfg.F), np.float32)
        HCb = cfg.HPG * DH
        for j, h in enumerate(hs):
            qrows = h * DH + _PERM
            krows = C + h * DH + _PERM
            vrows = 2 * C + h * DH + np.arange(DH)
            wq[:cfg.C, j * DH:(j + 1) * DH] = W_qkv[qrows, :].T
            wq[:cfg.C, HCb + j * DH:HCb + (j + 1) * DH] = W_qkv[krows, :].T
            wq[:cfg.C, 2 * HCb + j * DH:2 * HCb + (j + 1) * DH] = W_qkv[vrows, :].T
            wq[cfg.C, j * DH:(j + 1) * DH] = q_bias[h * DH + _PERM]
            wq[cfg.C, 2 * HCb + j * DH:2 * HCb + (j + 1) * DH] = v_bias[h * DH:(h + 1) * DH]

        cols = np.concatenate([np.arange(h * DH, (h + 1) * DH) for h in hs])
        wpT = W_proj[:, cols].T.copy()                      # (HC, C)

        in_maps.append({
            "xmT": xmT,
            "wqkvT": wq.astype(bf),
            "ropeC": ropeC,
            "ropeS": ropeS,
            "wpT": wpT.astype(bf),
            "scales": scale[hs].reshape(1, cfg.HPG).astype(np.float32),
            "ident": np.eye(128, dtype=np.float32).astype(bf),
        })
    return in_maps


def kernel(x, cached_x, W_qkv, q_bias, v_bias, W_proj, b_proj,
           scale_mul_log, rope_grid, num_remain):
    n = int(num_remain)
    assert n == NREM, f"kernel compiled for num_remain={NREM}, got {n}"
    x = np.asarray(x, np.float32)
    cached_x = np.asarray(cached_x, np.float32)
    b_proj = np.asarray(b_proj, np.float32)

    idx = _topk_idx(x, n)
    cfg = FULL_CFG
    in_maps = make_in_maps(x, cached_x, W_qkv, q_bias, v_bias, W_proj, b_proj,
                           scale_mul_log, rope_grid, idx, cfg)
    has_bias = bool(np.any(np.asarray(q_bias)) or np.any(np.asarray(v_bias)))
    nc = build_full_program(has_bias=has_bias)
    res = bass_utils.run_bass_kernel_spmd(
        nc, in_maps, core_ids=list(range(N_CORES)))
    outs = [np.asarray(r["outp"], np.float32) for r in res.results]

    n_groups = NH // cfg.HPG
    o_full = np.zeros((B, n, C), np.float32)
    for b in range(B):
        acc = outs[b * n_groups][:n]
        for g in range(1, n_groups):
            acc = acc + outs[b * n_groups + g][:n]
        o_full[b] = acc + b_proj

    up = np.broadcast_to(
        cached_x[:, :, None, :, None, :], (B, 32, 2, 32, 2, C)
    ).reshape(B, L, C)
    out = x + up
    bix = np.arange(B)[:, None]
    out[bix, idx] = x[bix, idx] + o_full
    return out.astype(np.float32)


# revision 37
# speedup vs baseline: 1.0165x; 1.0165x over previous
"""FastVAR cross-attention block kernel for 8 Trainium2 NeuronCores.

Sharding: 2 batches x 4 head-groups (4 heads each) = 8 cores (SPMD).

Per-core device program (v3 — engine-balanced, low DMA-dispatch count):
  phase 1 (per 128-token chunk; one 3D-AP DMA per chunk):
    qkv matmul (bf16, fp32 psum); norms/normalize read PSUM directly on DVE
    RoPE in split-halves layout (even|odd): 4 muls on GpSimd, 2 combines on DVE
    v(+ones) packed by ACT; q,k transposed on PE (+ACT copy) to feature-major
  phase 2 (attention, 8 units = 2 head-pairs x 4 query-quarters of 416):
    flat software pipeline across (unit, kb) steps: scores for step i+1 are
    emitted before AV of step i so the in-order PE queue never blocks ACT
    scores: 2 row-tiled concurrent K=64 matmuls -> one 2-bank PSUM tile
    exp: single ACT instruction over both heads (N=832) -> bf16 SBUF
    AV with ones-augmented V (M=65) accumulating over k-chunks in PSUM
    normalize: early PSUM->SBUF copy, fast reciprocal, one DRAM broadcast
    projection interleaved one half-chunk per step (PSUM-pool reuse)
Host: top-k selection (bitwise-matches reference on CPU jax), gather, weight
slicing (rope even|odd permutation folded into W_qkv), partial-sum reduce,
scatter + residual.
"""

import math
import os
import sys
from contextlib import ExitStack

import numpy as np

import concourse.bass as bass
import concourse.bacc as bacc
import concourse.tile as tile
from concourse import mybir
from concourse import bass_utils

# ---------------------------------------------------------------- constants
B = 2
L = 4096
C = 1024
NH = 16
DH = 64
NREM = 1638          # num_remain for this problem
NT = 1664            # padded token count (13 * 128)
HPG = 4              # heads per core (16 heads / 4 groups)
N_CORES = 8
QQ = 416             # query block (4 blocks of 416 = 1664)

F32 = mybir.dt.float32
BF16 = mybir.dt.bfloat16


class Cfg:
    def __init__(self, NT, NTR, C, HPG, DH, has_bias=True):
        self.NT, self.NTR, self.C, self.HPG, self.DH = NT, NTR, C, HPG, DH
        self.has_bias = has_bias
        self.NC = NT // 128          # token chunks
        self.CH = C // 128           # contraction chunks
        self.F = 3 * HPG * DH        # qkv feature width (768)
        self.HC = HPG * DH           # head channels per core (256)
        self.HCC = self.HC // 128    # proj contraction chunks (2)


FULL_CFG = Cfg(NT=NT, NTR=NREM, C=C, HPG=HPG, DH=DH)


# ---------------------------------------------------------------- device IR
def emit_core_program(tc, outs, ins, cfg):
    nc = tc.nc
    NTc, NC, CH = cfg.NT, cfg.NC, cfg.CH
    F = cfg.F
    X = mybir.AxisListType.X
    AF = mybir.ActivationFunctionType

    xmT, wqkvT = ins["xmT"], ins["wqkvT"]
    ropeC, ropeS = ins["ropeC"], ins["ropeS"]
    wpT, scales = ins["wpT"], ins["scales"]
    outp = outs["outp"]

    with ExitStack() as ctx:
        const = ctx.enter_context(tc.tile_pool(name="const", bufs=1))

        # xm per-chunk tiles; one 3D DMA per token chunk (chunk-level deps)
        xm_c = [const.tile([128, CH, 128], BF16, name=f"xmc{t}",
                           tag=f"xmc{t}") for t in range(NC)]
        xmT_r = xmT[0:cfg.C, :].rearrange("(c p) t -> p c t", p=128)
        # first two chunks before the weights so chunk 0 can start early
        for t in range(2):
            nc.sync.dma_start(xm_c[t][:], xmT_r[:, :, t * 128:(t + 1) * 128])

        # ---- resident input tiles ----------------------------------------
        w_t = []
        for ci in range(CH):
            t = const.tile([128, F], BF16, tag=f"w{ci}")
            nc.sync.dma_start(t[:], wqkvT[ci * 128:(ci + 1) * 128, :])
            w_t.append(t)
        w_bias = const.tile([1, F], BF16, tag="wb")
        nc.sync.dma_start(w_bias[:], wqkvT[cfg.C:cfg.C + 1, :])
        wp_t = []
        for hc in range(cfg.HCC):
            t = const.tile([128, cfg.C], BF16, tag=f"wp{hc}")
            nc.sync.dma_start(t[:], wpT[hc * 128:(hc + 1) * 128, :])
            wp_t.append(t)
        s_t = const.tile([128, cfg.HPG], F32, tag="scales")
        nc.sync.dma_start(s_t[:], scales[0:1, :].to_broadcast((128, cfg.HPG)))
        # rope tables, chunk-major: [128, NC, DH//2] (one DMA each)
        rct = const.tile([128, NC, DH // 2], BF16, tag="rct")
        rst = const.tile([128, NC, DH // 2], BF16, tag="rst")
        nc.sync.dma_start(rct[:], ropeC.rearrange("(c p) d -> p c d", p=128))
        nc.sync.dma_start(rst[:], ropeS.rearrange("(c p) d -> p c d", p=128))
        ones_row = const.tile([1, NTc], BF16, tag="ones_row")
        nc.sync.dma_start(ones_row[:], xmT[cfg.C:cfg.C + 1, :])
        ident = const.tile([128, 128], BF16, tag="ident")
        nc.sync.dma_start(ident[:], ins["ident"][:])
        eps_t = const.tile([128, 1], F32, tag="eps")
        nc.vector.memset(eps_t[:], 1e-12)

        # feature-major q,k (pairs of heads stacked 64+64)
        qkT = const.tile([128, 4, NTc], BF16, tag="qkT")
        vav = const.tile([128, NC, cfg.HPG, DH + 1], BF16, tag="vav")
        oPair = [const.tile([128, NTc], BF16, name=f"oP{i}", tag=f"oP{i}")
                 for i in range(2)]

        wk = ctx.enter_context(tc.tile_pool(name="wk", bufs=2))
        gp = ctx.enter_context(tc.tile_pool(name="gp", bufs=2))
        pe = ctx.enter_context(tc.tile_pool(name="exp", bufs=3))
        ms = ctx.enter_context(tc.tile_pool(name="ms", bufs=2))

        # ones column of vav is constant: set once (zero pad rows of last chunk)
        pad0 = cfg.NTR - (NC - 1) * 128
        nc.vector.memset(vav[:, 0:NC - 1, :, DH:DH + 1], 1.0)
        nc.vector.memset(vav[:, NC - 1, :, DH:DH + 1], 0.0)
        nc.vector.memset(vav[0:pad0, NC - 1, :, DH:DH + 1], 1.0)

        # ---------------- phase 1: qkv + norm + rope + transposes ----------
        with tc.tile_pool(name="p1ps", bufs=3, space="PSUM") as p1, \
             tc.tile_pool(name="tpps", bufs=2, space="PSUM") as tp:
            tp_pending = []

            def drain_tp(final=False):
                # keep the newest entry queued: its qkr is still ~1.4us from
                # ready when this chunk's matmuls end (2-chunk deferral)
                todo = tp_pending if final else tp_pending[:-1]
                for dsl, src in todo:
                    tps = tp.tile([128, 4, 128], BF16, name="tps", tag="tps")
                    for j in range(4):
                        nc.tensor.transpose(
                            tps[:, j, :], src[:, j * 128:(j + 1) * 128],
                            ident[:])
                    nc.scalar.copy(qkT[:, :, dsl], tps[:])
                tp_pending[:] = [] if final else tp_pending[-1:]

            for t in range(NC):
                tsl = slice(t * 128, (t + 1) * 128)
                if t + 2 < NC:
                    t2 = t + 2
                    nc.sync.dma_start(xm_c[t2][:],
                                      xmT_r[:, :, t2 * 128:(t2 + 1) * 128])

                ps = p1.tile([128, F], F32)
                n_ci = CH + 1 if cfg.has_bias else CH
                for ci in range(n_ci):
                    lhs = xm_c[t][:, ci, :] if ci < CH else ones_row[:, tsl]
                    rhsw = w_t[ci] if ci < CH else w_bias
                    for n0 in range(0, F, 512):
                        nn = min(512, F - n0)
                        nc.tensor.matmul(
                            ps[:, n0:n0 + nn], lhs, rhsw[:, n0:n0 + nn],
                            start=(ci == 0), stop=(ci == n_ci - 1))
                # previous chunk's transposes go behind this chunk's matmuls
                # so the PE never waits on the norm/rope chain
                drain_tp()

                # l2 norms over dh for the 8 q+k head-groups (PSUM-direct)
                sq = wk.tile([128, 512], BF16, tag="sq")
                nc.scalar.square(sq[:], ps[:, 0:512])
                ss = wk.tile([128, 8], F32, tag="ss")
                nc.vector.reduce_sum(
                    ss[:], sq.rearrange("p (h d) -> p h d", d=DH), axis=X)
                sroot = wk.tile([128, 8], F32, tag="sroot")
                nc.scalar.activation(sroot[:], ss[:], AF.Sqrt,
                                     bias=eps_t[:])
                rr8 = wk.tile([128, 8, 1], F32, tag="rr8")
                nc.vector.reciprocal(
                    rr8.rearrange("p h one -> p (h one)"), sroot[:])
                rrq = wk.tile([128, cfg.HPG, 1], F32, tag="rrq")
                nc.vector.tensor_mul(
                    rrq.rearrange("p h one -> p (h one)"),
                    rr8[:, 0:cfg.HPG, 0], s_t[:])

                # normalize q (with head scale) and k, direct from PSUM
                qkn = wk.tile([128, 8, DH], BF16, tag="qkn")
                nc.vector.tensor_mul(
                    qkn[:, 0:4, :],
                    ps[:, 0:256].rearrange("p (h d) -> p h d", d=DH),
                    rrq.to_broadcast((128, 4, DH)))
                nc.vector.tensor_mul(
                    qkn[:, 4:8, :],
                    ps[:, 256:512].rearrange("p (h d) -> p h d", d=DH),
                    rr8[:, 4:8, :].to_broadcast((128, 4, DH)))

                # v -> vav (ACT copy, PSUM source)
                nc.scalar.copy(
                    vav[:, t, :, 0:DH],
                    ps[:, 512:768].rearrange("p (h d) -> p h d", d=DH))

                # rope (dh layout = [even32 | odd32] per head): muls on gpsimd
                DH2 = DH // 2
                rc_b = rct[:, t:t + 1, :].to_broadcast((128, 8, DH2))
                rs_b = rst[:, t:t + 1, :].to_broadcast((128, 8, DH2))
                m_ce = gp.tile([128, 8, DH2], BF16, tag="m_ce")
                m_so = gp.tile([128, 8, DH2], BF16, tag="m_so")
                m_se = gp.tile([128, 8, DH2], BF16, tag="m_se")
                m_co = gp.tile([128, 8, DH2], BF16, tag="m_co")
                nc.gpsimd.tensor_mul(m_ce[:], qkn[:, :, 0:DH2], rc_b)
                nc.gpsimd.tensor_mul(m_so[:], qkn[:, :, DH2:DH], rs_b)
                nc.gpsimd.tensor_mul(m_se[:], qkn[:, :, 0:DH2], rs_b)
                nc.gpsimd.tensor_mul(m_co[:], qkn[:, :, DH2:DH], rc_b)
                qkr = wk.tile([128, 8, DH], BF16, tag="qkr", bufs=3)
                nc.vector.tensor_sub(qkr[:, :, 0:DH2], m_ce[:], m_so[:])
                nc.vector.tensor_add(qkr[:, :, DH2:DH], m_se[:], m_co[:])

                # feature-major q,k via PE transpose + ACT copy (deferred)
                qkr2 = qkr.rearrange("p h d -> p (h d)")
                tp_pending.append((tsl, qkr2))
            drain_tp(final=True)

        # ---------------- phase 2: attention + interleaved projection ------
        # 3 main query blocks of 512 (bank-exact) + one shared 128-query tail
        # unit covering all 4 heads. Scores for step i+1 are emitted before AV
        # of step i; projection drains one matmul per step.
        QB = 512
        QT0 = 3 * QB                       # tail start (1536)
        steps = [("m", qq, pair, kb)
                 for qq in range(3) for pair in range(2) for kb in range(NC)]
        steps += [("t", 0, 0, kb) for kb in range(NC)]

        with tc.tile_pool(name="scps", bufs=2, space="PSUM") as scp, \
             tc.tile_pool(name="otps", bufs=1, space="PSUM") as otp, \
             tc.tile_pool(name="pjps", bufs=2, space="PSUM") as pjp, \
             tc.tile_pool(name="dscr", bufs=4, space="DRAM") as pd:

            sc_tiles = {}
            oT_cur = [None, None]
            proj_pending = []   # single-matmul granularity
            proj_state = {}

            def emit_scores(step):
                kind, qq, pair, kb = step
                ksl = slice(kb * 128, (kb + 1) * 128)
                scP = scp.tile([128, 2, QB], F32, tag="sc")
                if kind == "m":
                    qsl = slice(qq * QB, (qq + 1) * QB)
                    for i in range(2):
                        nc.tensor.matmul(
                            scP[:, i, :],
                            qkT[i * DH:(i + 1) * DH, 2 + pair, ksl],
                            qkT[i * DH:(i + 1) * DH, pair, qsl],
                            start=True, stop=True)
                else:
                    # row-tile index i selects the bank so the two concurrent
                    # matmuls never write the same PSUM bank
                    for h in range(4):
                        p2_, i = h // 2, h % 2
                        nc.tensor.matmul(
                            scP[:, i, p2_ * 256:p2_ * 256 + 128],
                            qkT[i * DH:(i + 1) * DH, 2 + p2_, ksl],
                            qkT[i * DH:(i + 1) * DH, p2_, QT0:NTc],
                            start=True, stop=True)
                sc_tiles[step] = scP

            def emit_proj_one(end_drain=False):
                """Emit a single projection matmul (plus copy+store when a
                half-chunk completes)."""
                if not proj_pending:
                    return
                t, half = proj_pending[0]
                hc = proj_state.get((t, half), 0)
                tsl = slice(t * 128, (t + 1) * 128)
                nsl = slice(half * 512, (half + 1) * 512)
                if hc == 0:
                    psp = pjp.tile([128, 512], F32, tag="pj")
                    proj_state[(t, half, "ps")] = psp
                else:
                    psp = proj_state.pop((t, half, "ps"))
                nc.tensor.matmul(
                    psp[:], oPair[hc][:, tsl], wp_t[hc][:, nsl],
                    start=(hc == 0), stop=(hc == cfg.HCC - 1))
                if hc == cfg.HCC - 1:
                    ob = ms.tile([128, 512], BF16, tag="ob")
                    if end_drain:
                        # ACT is idle after the last exp; the DVE FIFO is
                        # blocked behind the tail-unit reciprocal's DMA wait
                        nc.scalar.copy(ob[:], psp[:])
                    else:
                        nc.vector.tensor_copy(ob[:], psp[:])
                    nc.gpsimd.dma_start(outp[tsl, nsl], ob[:])
                    proj_pending.pop(0)
                    proj_state.pop((t, half), None)
                else:
                    proj_state[(t, half)] = hc + 1

            def normalize_main(pair, qq):
                qsl = slice(qq * QB, (qq + 1) * QB)
                oTs = []
                for i in range(2):
                    t_sb = ms.tile([DH + 1, QB], F32, name=f"oTs{i}",
                                   tag=f"oTs{i}")
                    nc.vector.tensor_copy(t_sb[:], oT_cur[i][:])
                    oTs.append(t_sb)
                den_d = pd.tile([1, 2 * QB], F32, name="den_d", tag="den_d")
                for i in range(2):
                    nc.sync.dma_start(den_d[0:1, i * QB:(i + 1) * QB],
                                      oTs[i][DH:DH + 1, :])
                NP8 = 2 * QB // 8
                dshuf = bass.AP(tensor=den_d.tensor, offset=den_d.offset,
                                ap=[[8, NP8], [1, 8]])
                dt = ms.tile([NP8, 8], F32, tag="dt")
                nc.sync.dma_start(dt[:], dshuf)
                rt = ms.tile([NP8, 8], F32, tag="rt")
                nc.vector.reciprocal(rt[:], dt[:])
                rec_d = pd.tile([1, 2 * QB], F32, name="rec_d", tag="rec_d")
                rshuf = bass.AP(tensor=rec_d.tensor, offset=rec_d.offset,
                                ap=[[8, NP8], [1, 8]])
                nc.sync.dma_start(rshuf, rt[:])
                reb = ms.tile([DH, 2, QB], F32, tag="reb")
                bc_src = bass.AP(tensor=rec_d.tensor, offset=rec_d.offset,
                                 ap=[[0, DH], [1, 2 * QB]])
                nc.sync.dma_start(
                    reb.rearrange("p two q -> p (two q)"), bc_src)
                for i in range(2):
                    nc.vector.tensor_mul(
                        oPair[pair][i * DH:(i + 1) * DH, qsl],
                        oTs[i][0:DH, :], reb[:, i, :])

            def normalize_tail():
                oTs = []
                for i in range(2):
                    t_sb = ms.tile([DH + 1, QB], F32, name=f"oTs{i}",
                                   tag=f"oTs{i}")
                    nc.vector.tensor_copy(t_sb[:], oT_cur[i][:])
                    oTs.append(t_sb)
                den_d = pd.tile([1, 512], F32, name="den_dt", tag="den_dt")
                for p2_ in range(2):
                    src3 = oTs[p2_][DH:DH + 1, :].rearrange(
                        "one (j x) -> one j x", j=2)[:, :, 0:128]
                    dst3 = den_d[0:1, p2_ * 256:(p2_ + 1) * 256].rearrange(
                        "one (j x) -> one j x", x=128)
                    nc.sync.dma_start(dst3, src3)
                NP8 = 64
                dshuf = bass.AP(tensor=den_d.tensor, offset=den_d.offset,
                                ap=[[8, NP8], [1, 8]])
                dt = ms.tile([NP8, 8], F32, tag="dtt")
                nc.sync.dma_start(dt[:], dshuf)
                rt = ms.tile([NP8, 8], F32, tag="rtt")
                nc.vector.reciprocal(rt[:], dt[:])
                rec_d = pd.tile([1, 512], F32, name="rec_dt", tag="rec_dt")
                rshuf = bass.AP(tensor=rec_d.tensor, offset=rec_d.offset,
                                ap=[[8, NP8], [1, 8]])
                nc.sync.dma_start(rshuf, rt[:])
                reb = ms.tile([DH, 512], F32, tag="rebt")
                bc_src = bass.AP(tensor=rec_d.tensor, offset=rec_d.offset,
                                 ap=[[0, DH], [1, 512]])
                nc.sync.dma_start(reb[:], bc_src)
                for h in range(4):
                    p2_, i = h // 2, h % 2
                    nc.vector.tensor_mul(
                        oPair[p2_][i * DH:(i + 1) * DH, QT0:NTc],
                        oTs[p2_][0:DH, i * 256:i * 256 + 128],
                        reb[:, h * 128:(h + 1) * 128])

            emit_scores(steps[0])
            for si, step in enumerate(steps):
                kind, qq, pair, kb = step
                if kb == 0:
                    oT_cur = [otp.tile([DH + 1, QB], F32, name=f"oT{i}",
                                       tag=f"oT{i}") for i in range(2)]
                if si + 1 < len(steps):
                    emit_scores(steps[si + 1])
                scP = sc_tiles.pop(step)
                if kind == "m":
                    exb = pe.tile([128, 2, QB], BF16, tag="ex")
                    nc.scalar.activation(exb[:], scP[:], AF.Exp)
                    for i in range(2):
                        h = pair * 2 + i
                        nc.tensor.matmul(
                            oT_cur[i][:], vav[:, kb, h, :], exb[:, i, :],
                            start=(kb == 0), stop=(kb == NC - 1))
                else:
                    sc4 = scP.rearrange("p two (j x) -> p two j x",
                                        j=2)[:, :, :, 0:128]
                    exb = pe.tile([128, 2, 2, 128], BF16, tag="ext")
                    nc.scalar.activation(exb[:], sc4, AF.Exp)
                    # start=True clears the WHOLE bank's has_written bits,
                    # so only the first head writing each oT bank may set it
                    for h in range(4):
                        p2_, i = h // 2, h % 2
                        nc.tensor.matmul(
                            oT_cur[p2_][:, i * 256:i * 256 + 128],
                            vav[:, kb, h, :], exb[:, i, p2_, :],
                            start=(kb == 0 and i == 0),
                            stop=(kb == NC - 1))
                emit_proj_one()
                if kb == NC - 1:
                    if kind == "m":
                        normalize_main(pair, qq)
                        if pair == 1:
                            for t in range(4 * qq, 4 * qq + 4):
                                proj_pending.append((t, 0))
                                proj_pending.append((t, 1))
                    else:
                        # flush leftovers first: their DVE copies must not
                        # queue behind the tail reciprocal's DMA round-trip
                        while proj_pending:
                            emit_proj_one(end_drain=True)
                        normalize_tail()
                        proj_pending.append((12, 0))
                        proj_pending.append((12, 1))
            while proj_pending:
                emit_proj_one(end_drain=True)


# ---------------------------------------------------------------- build
def declare_io(nc, cfg):
    ins = {
        "xmT": nc.dram_tensor("xmT", [cfg.C + 1, cfg.NT], BF16,
                              kind="ExternalInput").ap(),
        "wqkvT": nc.dram_tensor("wqkvT", [cfg.C + 1, cfg.F], BF16,
                                kind="ExternalInput").ap(),
        "ropeC": nc.dram_tensor("ropeC", [cfg.NT, cfg.DH // 2], BF16,
                                kind="ExternalInput").ap(),
        "ropeS": nc.dram_tensor("ropeS", [cfg.NT, cfg.DH // 2], BF16,
                                kind="ExternalInput").ap(),
        "wpT": nc.dram_tensor("wpT", [cfg.HC, cfg.C], BF16,
                              kind="ExternalInput").ap(),
        "scales": nc.dram_tensor("scales", [1, cfg.HPG], F32,
                                 kind="ExternalInput").ap(),
        "ident": nc.dram_tensor("ident", [128, 128], BF16,
                                kind="ExternalInput").ap(),
    }
    outs = {
        "outp": nc.dram_tensor("outp", [cfg.NT, cfg.C], BF16,
                               kind="ExternalOutput").ap(),
    }
    return ins, outs


_BUILD_CACHE = {}

if os.environ.get("LDW_OPT", "0") == "1":
    _orig_run_command = bass_utils.run_command

    def _patched_run_command(argv, **kw):
        argv = ["--enable-ldw-opt=true" if a == "--enable-ldw-opt=false" else a
                for a in argv]
        return _orig_run_command(argv, **kw)

    bass_utils.run_command = _patched_run_command


def build_full_program(has_bias=True):
    key = ("full", has_bias)
    if key in _BUILD_CACHE:
        return _BUILD_CACHE[key]
    cfg = Cfg(NT=NT, NTR=NREM, C=C, HPG=HPG, DH=DH, has_bias=has_bias)
    nc = bacc.Bacc("TRN2", target_bir_lowering=False, debug=False,
                   num_devices=N_CORES)
    ins, outs = declare_io(nc, cfg)
    with tile.TileContext(nc) as tc:
        emit_core_program(tc, outs, ins, cfg)
    nc.compile()
    _BUILD_CACHE[key] = nc
    return nc


# ---------------------------------------------------------------- host side
def _topk_idx(x, n):
    """Replicate reference token selection exactly (CPU jax; numpy fallback)."""
    try:
        import jax
        import jax.numpy as jnp
        cpu = jax.devices("cpu")[0]
        with jax.default_device(cpu):
            xj = jax.device_put(np.asarray(x), cpu)
            mean = jnp.mean(xj, axis=1, keepdims=True)
            mse = jnp.sum((xj - mean) ** 2, axis=-1)
            idx = jnp.argsort(-mse, axis=1)[:, :n]
            return np.asarray(idx)
    except Exception:
        x = np.asarray(x, np.float32)
        mean = x.mean(1, keepdims=True, dtype=np.float32)
        mse = ((x - mean) ** 2).sum(-1, dtype=np.float32)
        return np.argsort(-mse, axis=1, kind="stable")[:, :n]


# rope even|odd permutation of a head's 64 features
_PERM = np.concatenate([np.arange(0, DH, 2), np.arange(1, DH, 2)])


def make_in_maps(x, cached_x, W_qkv, q_bias, v_bias, W_proj, b_proj,
                 scale_mul_log, rope_grid, idx, cfg):
    x = np.asarray(x, np.float32)
    W_qkv = np.asarray(W_qkv, np.float32)
    W_proj = np.asarray(W_proj, np.float32)
    q_bias = np.asarray(q_bias, np.float32)
    v_bias = np.asarray(v_bias, np.float32)
    rope_grid = np.asarray(rope_grid, np.float32)
    scale = np.exp(np.minimum(np.asarray(scale_mul_log, np.float32),
                              math.log(100.0))).reshape(NH)

    n_groups = NH // cfg.HPG
    in_maps = []
    per_batch = {}
    import ml_dtypes
    bf = ml_dtypes.bfloat16
    for b in range(B):
        xm = x[b, idx[b]]                                   # (NREM, C)
        xmT = np.zeros((cfg.C + 1, cfg.NT), np.float32)
        xmT[:cfg.C, :cfg.NTR] = xm.T
        xmT[cfg.C, :cfg.NTR] = 1.0
        ropeC = np.zeros((cfg.NT, DH // 2), np.float32)
        ropeS = np.zeros((cfg.NT, DH // 2), np.float32)
        ropeC[:cfg.NTR] = rope_grid[0][idx[b]]
        ropeS[:cfg.NTR] = rope_grid[1][idx[b]]
        per_batch[b] = (xmT.astype(bf), ropeC.astype(bf), ropeS.astype(bf))

    for core in range(N_CORES):
        b, hg = divmod(core, n_groups)
        hs = list(range(hg * cfg.HPG, (hg + 1) * cfg.HPG))
        xmT, ropeC, ropeS = per_batch[b]

        wq = np.zeros((cfg.C + 1, cfg.F), np.float32)
        HCb = cfg.HPG * DH
        for j, h in enumerate(hs):
            qrows = h * DH + _PERM
            krows = C + h * DH + _PERM
            vrows = 2 * C + h * DH + np.arange(DH)
            wq[:cfg.C, j * DH:(j + 1) * DH] = W_qkv[qrows, :].T
            wq[:cfg.C, HCb + j * DH:HCb + (j + 1) * DH] = W_qkv[krows, :].T
            wq[:cfg.C, 2 * HCb + j * DH:2 * HCb + (j + 1) * DH] = W_qkv[vrows, :].T
            wq[cfg.C, j * DH:(j + 1) * DH] = q_bias[h * DH + _PERM]
            wq[cfg.C, 2 * HCb + j * DH:2 * HCb + (j + 1) * DH] = v_bias[h * DH:(h + 1) * DH]

        cols = np.concatenate([np.arange(h * DH, (h + 1) * DH) for h in hs])
        wpT = W_proj[:, cols].T.copy()                      # (HC, C)

        in_maps.append({
            "xmT": xmT,
            "wqkvT": wq.astype(bf),
            "ropeC": ropeC,
            "ropeS": ropeS,
            "wpT": wpT.astype(bf),
            "scales": scale[hs].reshape(1, cfg.HPG).astype(np.float32),
            "ident": np.eye(128, dtype=np.float32).astype(bf),
        })
    return in_maps


def kernel(x, cached_x, W_qkv, q_bias, v_bias, W_proj, b_proj,
           scale_mul_log, rope_grid, num_remain):
    n = int(num_remain)
    assert n == NREM, f"kernel compiled for num_remain={NREM}, got {n}"
    x = np.asarray(x, np.float32)
    cached_x = np.asarray(cached_x, np.float32)
    b_proj = np.asarray(b_proj, np.float32)

    idx = _topk_idx(x, n)
    cfg = FULL_CFG
    in_maps = make_in_maps(x, cached_x, W_qkv, q_bias, v_bias, W_proj, b_proj,
                           scale_mul_log, rope_grid, idx, cfg)
    has_bias = bool(np.any(np.asarray(q_bias)) or np.any(np.asarray(v_bias)))
    nc = build_full_program(has_bias=has_bias)
    res = bass_utils.run_bass_kernel_spmd(
        nc, in_maps, core_ids=list(range(N_CORES)))
    outs = [np.asarray(r["outp"], np.float32) for r in res.results]

    n_groups = NH // cfg.HPG
    o_full = np.zeros((B, n, C), np.float32)
    for b in range(B):
        acc = outs[b * n_groups][:n]
        for g in range(1, n_groups):
            acc = acc + outs[b * n_groups + g][:n]
        o_full[b] = acc + b_proj

    up = np.broadcast_to(
        cached_x[:, :, None, :, None, :], (B, 32, 2, 32, 2, C)
    ).reshape(B, L, C)
    out = x + up
    bix = np.arange(B)[:, None]
    out[bix, idx] = x[bix, idx] + o_full
    return out.astype(np.float32)
